# revision 43
# baseline (speedup 1.0000x reference)
"""Trainium2 Bass kernel for AxisLengthNetMetric (chamfer-distance + L1-size metric).

Reference computation (per row n of N = 262144):
  gt_box row -> size (cols 3:6), rx (6:9), ry (9:12)
  rx_hat = rx/|rx|, ry_hat = ry/|ry|, rz = cross(rx_hat, ry_hat)
  corners u_c = sum_k sign[c,k] * 0.5*size[k] * axis_k   (8 corners, +-pairs)
  chamfer(corners, pred_pts[n]): d[p,q] = |a_p - b_q|^2, dist1 = min_q, dist2 = min_p
  out[0] = mean over (N,8) of dist1+dist2 ; out[1] = mean |size - pred_size|

Kernel strategy (v5):
- data parallel over 8 cores; per core 32768 rows as 128 partitions x 256.
- 4 distinct corners up to sign (u' = 2u prescaled, g' = u'.b):
    dist1 sums: min_q(b2 -+ g'), dist2: b2_q + min_i(a2_i - |g'|);
    a2/b2/min contributions accumulated via ACT accum_out, host-combined.
- fp16 end to end (inputs DMA'd as fp16): every TensorTensor with all
  operands 2-byte and packed-innermost runs in DVE 2x mode (0.52 ns/elem).
- dot products as ONE DVE mul in [i,q,d] layout (d innermost => both
  broadcast operands stay packed => 2x), then the d-sum (the only work the
  gpsimd engine gets) as two flattened-3D adds on Pool; everything else
  rides DVE (2x) or ACT (abs/squares/copies/accumulations, transposed
  writes for free axis swaps).
- walrus constraints honored: no ScalarTensorTensor on Pool, no gpsimd
  min, no tensor_tensor_reduce; ScalarTensorTensor APs <= 3D.
- phase-1 software-pipelined (5-stage skew) over tiles of [64,64,64,48,16]
  rows: the shrinking tail tiles shorten the end-of-kernel serial chain
  (last mt -> d-sum -> eeb -> trees -> accum) by ~2.5x. phase-0 in 2
  chunks (scalar-chain / combos sub-stages) overlapped with the tile
  stream; e1 and t2 share one ET tile so their pair-min is a single op.
- engine busy: DVE ~52us, Pool ~34us, ACT ~35us; TimelineSim ~65.5us/core
  (v4 baseline: 81.9us).
"""

import numpy as np

import concourse.bacc as bacc
import concourse.bass as bass  # noqa: F401
import concourse.tile as tile
from concourse import mybir

F32 = mybir.dt.float32
F16 = mybir.dt.float16
ALU = mybir.AluOpType
ACTF = mybir.ActivationFunctionType
AX = mybir.AxisListType

P = 128
N_CORES = 8
N_TOTAL = 262144
NC_N = N_TOTAL // N_CORES  # 32768 rows per core
G_PROD = 64                # rows per partition per heavy tile
# engine assignment knobs: True = put op on gpsimd (Pool)
KNOB = {
    "wtutc_gp": False, "xe_bufs": 2, "order": "rev", "p0k": 1, "nchunk": 2,
    "seq": False, "dsum": "pp", "p0_gp": False, "l1d_gp": False,
    "eebp_gp": False, "b2_gp": False, "t2b_gp": False, "sched": "B",
}

# accT slots per tile
MINSUM, SQA, SQB, L1, T2SUM = 0, 1, 2, 3, 4
NSLOT = 5


TILE_SIZES = [64, 64, 64, 48, 16]  # shrinking tail shortens the drain chain


def build_nc(nc_n=NC_N, G=G_PROD, tile_sizes=None):
    GA = nc_n // P             # all rows per partition
    if tile_sizes is None:
        tile_sizes = TILE_SIZES if GA == 256 else [G] * (GA // G)
    assert sum(tile_sizes) == GA
    TILES = []
    r = 0
    for gsz in tile_sizes:
        TILES.append((r, gsz))
        r += gsz
    ntiles = len(TILES)

    nc = bacc.Bacc("TRN2", target_bir_lowering=False, debug=False)

    gt = nc.dram_tensor("gt", [nc_n, 12], F16, kind="ExternalInput").ap()
    pred = nc.dram_tensor("pred", [nc_n, 24], F16, kind="ExternalInput").ap()
    ps = nc.dram_tensor("ps", [nc_n, 3], F16, kind="ExternalInput").ap()
    out = nc.dram_tensor("out", [P, ntiles * NSLOT], F32, kind="ExternalOutput").ap()

    gt_r = gt.rearrange("(p g) f -> p g f", p=P)
    pred_r = pred.rearrange("(p g) f -> p g f", p=P)
    ps_r = ps.rearrange("(p g) f -> p g f", p=P)

    def gp_tt(out_, a, b, op):
        # plain TensorTensor on gpsimd (walrus rejects TensorScalarPtr on Pool)
        nc.gpsimd.tensor_tensor(out_, a, b, op=op)

    with tile.TileContext(nc) as tc:
        with (
            tc.tile_pool(name="per", bufs=1) as per,   # persistent / phase-0
            tc.tile_pool(name="io", bufs=3) as io,
            tc.tile_pool(name="scr", bufs=1) as scr,
            tc.tile_pool(name="xe", bufs=KNOB["xe_bufs"]) as xe,
        ):
            accT = per.tile([P, ntiles, NSLOT], F32)
            # SQA only fills NCHUNK of the ntiles slots - zero the rest
            nc.gpsimd.memset(accT, 0.0)

            # warm the ACT function tables before any data dependency
            warm = per.tile([P, 2], F32)
            nc.vector.memset(warm, 1.0)
            for fn in (ACTF.Sqrt, ACTF.Square, ACTF.Abs, ACTF.Identity):
                nc.scalar.activation(warm[:, 0:1], warm[:, 1:2], fn)

            # ================= phase 0: corner basis (pipelined stage) ========
            gta = per.tile([P, GA, 12], F16)
            uta = per.tile([P, GA, 4, 3], F16)
            a2ba = per.tile([P, GA, 4], F16)
            CH = KNOB.get("chunks", "half")
            if CH == "half":
                CHUNKS = [(0, GA // 2), (GA // 2, GA // 2)]
            elif CH == "64rest":
                CHUNKS = [(0, 64), (64, GA - 64)]
            elif CH == "3way":
                CHUNKS = [(0, 64), (64, 96), (160, GA - 160)]
            st0 = {}

            def P0a(c):
                r0, GC = CHUNKS[c]
                cs = slice(r0, r0 + GC)
                gtc = gta[:, cs]
                nc.sync.dma_start(out=gtc, in_=gt_r[:, cs])

                sqtT = scr.tile([P, GC, 3, 2], F16, tag="sqtT")
                nc.scalar.activation(
                    sqtT.transpose([0, 1, 3, 2]),
                    gtc[:, :, 6:12].rearrange("p g (v d) -> p g v d", d=3),
                    ACTF.Square,
                )
                n2a = scr.tile([P, GC, 2], F16, tag="n2a")
                n2t = scr.tile([P, GC, 2], F16, tag="n2t")
                nc.vector.tensor_add(n2a, sqtT[:, :, 0, :], sqtT[:, :, 1, :])
                nc.vector.tensor_add(n2t, n2a, sqtT[:, :, 2, :])
                srt = scr.tile([P, GC, 2], F16, tag="srt")
                nc.scalar.activation(srt, n2t, ACTF.Sqrt)  # |r|
                ivt = scr.tile([P, GC, 2], F16, tag="ivt")
                with nc.allow_low_precision(reason="fp16 1/|r|: 2e-2 rel-err budget"):
                    nc.vector.reciprocal(ivt, srt)         # 1/|r|
                cct = scr.tile([P, GC, 3], F16, tag="cct")
                nc.vector.tensor_mul(cct[:, :, 0:2], gtc[:, :, 3:5], ivt)
                tzt = scr.tile([P, GC, 1], F16, tag="tzt")
                nc.vector.tensor_mul(tzt, ivt[:, :, 0:1], ivt[:, :, 1:2])
                nc.vector.tensor_mul(cct[:, :, 2:3], gtc[:, :, 5:6], tzt)

                st0[c] = (gtc, cct)

            def P0b(c):
                r0, GC = CHUNKS[c]
                cs = slice(r0, r0 + GC)
                gtc, cct = st0[c]
                # cross product (raw rx x ry) on GPSIMD
                rxet = xe.tile([P, GC, 5], F16, tag="rxet")
                ryet = xe.tile([P, GC, 5], F16, tag="ryet")
                nc.scalar.copy(rxet[:, :, 0:3], gtc[:, :, 6:9])
                nc.scalar.copy(rxet[:, :, 3:5], gtc[:, :, 6:8])
                nc.scalar.copy(ryet[:, :, 0:3], gtc[:, :, 9:12])
                nc.scalar.copy(ryet[:, :, 3:5], gtc[:, :, 9:11])
                m1t = xe.tile([P, GC, 3], F16, tag="m1t")
                m2t = xe.tile([P, GC, 3], F16, tag="m2t")
                crt = xe.tile([P, GC, 3], F16, tag="crt")
                eng0 = nc.gpsimd if KNOB.get("p0_gp", True) else nc.vector
                def p0_tt(o_, a_, b_, op_):
                    if KNOB.get("p0_gp", True):
                        gp_tt(o_, a_, b_, op_)
                    else:
                        nc.vector.tensor_tensor(o_, a_, b_, op=op_)
                p0_tt(m1t, rxet[:, :, 1:4], ryet[:, :, 2:5], ALU.mult)
                p0_tt(m2t, rxet[:, :, 2:5], ryet[:, :, 1:4], ALU.mult)
                p0_tt(crt, m1t, m2t, ALU.subtract)

                # v01 = dirs * c01 ; v2 = cross * cz. The c scalars are
                # pre-expanded over d on ACT so the muls run in DVE 2x mode
                cce = xe.tile([P, GC, 3, 3], F16, tag="cce")
                nc.scalar.copy(cce, cct.unsqueeze(3).broadcast_to((P, GC, 3, 3)))
                v01t = xe.tile([P, GC, 2, 3], F16, tag="v01t")
                p0_tt(v01t[:, :, 0, :], gtc[:, :, 6:9], cce[:, :, 0], ALU.mult)
                p0_tt(v01t[:, :, 1, :], gtc[:, :, 9:12], cce[:, :, 1], ALU.mult)
                v2t = xe.tile([P, GC, 3], F16, tag="v2t")
                p0_tt(v2t, crt, cce[:, :, 2], ALU.mult)

                # u combos (DVE 2x fp16 packed, or Pool via knob)
                wt = scr.tile([P, GC, 2, 3], F16, tag="wt")
                utc = uta[:, cs]
                v2b = v2t.unsqueeze(2).broadcast_to((P, GC, 2, 3))
                if KNOB["wtutc_gp"]:
                    gp_tt(wt[:, :, 0, :], v01t[:, :, 0, :], v01t[:, :, 1, :], ALU.add)
                    gp_tt(wt[:, :, 1, :], v01t[:, :, 0, :], v01t[:, :, 1, :], ALU.subtract)
                    gp_tt(utc[:, :, 0:2, :], wt, v2b, ALU.add)
                    gp_tt(utc[:, :, 2:4, :], wt, v2b, ALU.subtract)
                else:
                    nc.vector.tensor_add(
                        wt[:, :, 0, :], v01t[:, :, 0, :], v01t[:, :, 1, :]
                    )
                    nc.vector.tensor_sub(
                        wt[:, :, 1, :], v01t[:, :, 0, :], v01t[:, :, 1, :]
                    )
                    nc.vector.tensor_add(utc[:, :, 0:2, :], wt, v2b)
                    nc.vector.tensor_sub(utc[:, :, 2:4, :], wt, v2b)

                # a2: squares (ACT, accum -> SQA slot), d-sum in fp16 2x
                squtT = xe.tile([P, GC, 3, 4], F16, tag="squtT")
                nc.scalar.activation(
                    squtT.transpose([0, 1, 3, 2]), uta[:, cs], ACTF.Square,
                    scale=0.5,  # (u'/2)^2 = u^2
                    accum_out=accT[:, c, SQA : SQA + 1],
                )
                a2s = scr.tile([P, GC, 4], F16, tag="a2s")
                nc.vector.tensor_add(a2s, squtT[:, :, 0, :], squtT[:, :, 1, :])
                nc.vector.tensor_add(a2ba[:, cs], a2s, squtT[:, :, 2, :])

            # ================= phase 1: pairwise chamfer, pipelined ===========
            # stage S1: DMA + dot mul; S2: d-sum (GP) + b2; S3: |g|/eeb/t2b;
            # S4: min-trees + fused sum. Emission interleaves tiles with skew
            # so each engine's in-order queue never waits on a cross-engine
            # producer that was emitted in the same stage.
            st = [dict() for _ in range(ntiles)]

            def S0(t):
                s = st[t]
                r0, Gt = TILES[t]
                sl = slice(r0, r0 + Gt)
                bt = io.tile([P, Gt, 8, 3], F16, tag="pred")
                pst = io.tile([P, Gt, 3], F16, tag="ps")
                nc.sync.dma_start(
                    out=bt, in_=pred_r[:, sl].rearrange("p g (q d) -> p g q d", d=3)
                )
                nc.sync.dma_start(out=pst, in_=ps_r[:, sl])
                s["bt"], s["pst"] = bt, pst

            def S1(t):
                s = st[t]
                r0, Gt = TILES[t]
                sl = slice(r0, r0 + Gt)
                bt = s["bt"]
                mt = xe.tile([P, Gt, 4, 8, 3], F16, tag="mt")
                ue = uta[:, sl].unsqueeze(3).broadcast_to((P, Gt, 4, 8, 3))
                be = bt.unsqueeze(2).broadcast_to((P, Gt, 4, 8, 3))
                nc.vector.tensor_mul(mt, ue, be)
                s["mt"] = mt
                s["mtf"] = mt.rearrange("p g i q d -> p (g i) q d")

            def S2(t):
                s = st[t]
                r0, Gt = TILES[t]
                bt = s["bt"]
                mtf = s["mtf"]
                mt = s["mt"]
                d01 = xe.tile([P, Gt, 4, 8], F16, tag="d01")
                d01f = d01.rearrange("p g i q -> p (g i) q")
                gb = xe.tile([P, Gt, 4, 8], F16, tag="gb")
                mode = KNOB.get("dsum", "pp")
                if mode == "pp":
                    gp_tt(d01f, mtf[:, :, :, 0], mtf[:, :, :, 1], ALU.add)
                    gp_tt(gb.rearrange("p g i q -> p (g i) q"), d01f,
                          mtf[:, :, :, 2], ALU.add)
                elif mode == "pd":   # d01 Pool, gb DVE
                    gp_tt(d01f, mtf[:, :, :, 0], mtf[:, :, :, 1], ALU.add)
                    nc.vector.tensor_add(gb, d01, mt[:, :, :, :, 2])
                elif mode == "dp":   # d01 DVE, gb Pool
                    nc.vector.tensor_add(d01, mt[:, :, :, :, 0], mt[:, :, :, :, 1])
                    gp_tt(gb.rearrange("p g i q -> p (g i) q"), d01f,
                          mtf[:, :, :, 2], ALU.add)
                elif mode == "split":  # d01 Pool; gb: half DVE, half Pool
                    gp_tt(d01f, mtf[:, :, :, 0], mtf[:, :, :, 1], ALU.add)
                    nc.vector.tensor_add(gb[:, :, 0:2], d01[:, :, 0:2],
                                         mt[:, :, 0:2, :, 2])
                    gp_tt(gb[:, :, 2:4], d01[:, :, 2:4], mt[:, :, 2:4, :, 2],
                          ALU.add)
                sqbtT = xe.tile([P, Gt, 3, 8], F16, tag="sqbtT")
                nc.scalar.activation(
                    sqbtT.transpose([0, 1, 3, 2]), bt, ACTF.Square,
                    accum_out=accT[:, t, SQB : SQB + 1],
                )
                b2s = scr.tile([P, Gt, 8], F16, tag="b2s")
                b2b = xe.tile([P, Gt, 8], F16, tag="b2b")
                if KNOB.get("b2_gp", False):
                    gp_tt(b2s, sqbtT[:, :, 0, :], sqbtT[:, :, 1, :], ALU.add)
                    gp_tt(b2b, b2s, sqbtT[:, :, 2, :], ALU.add)
                else:
                    nc.vector.tensor_add(b2s, sqbtT[:, :, 0, :], sqbtT[:, :, 1, :])
                    nc.vector.tensor_add(b2b, b2s, sqbtT[:, :, 2, :])
                s["gb"], s["b2b"] = gb, b2b

            def S3(t):
                s = st[t]
                r0, Gt = TILES[t]
                gb, b2b = s["gb"], s["b2b"]
                sl = slice(r0, r0 + Gt)
                agbT = xe.tile([P, Gt, 8, 4], F16, tag="agbT")
                nc.scalar.activation(agbT.transpose([0, 1, 3, 2]), gb, ACTF.Abs)
                b2bc = b2b.unsqueeze(2).broadcast_to((P, Gt, 4, 8))
                eeb = xe.tile([P, Gt, 8, 8], F16, tag="eeb")
                nc.vector.tensor_sub(eeb[:, :, 0:4, :], b2bc, gb)
                if KNOB.get("eebp_gp", False):
                    gp_tt(eeb[:, :, 4:8, :], b2bc, gb, ALU.add)
                else:
                    nc.vector.tensor_add(eeb[:, :, 4:8, :], b2bc, gb)
                a2bcT = a2ba[:, sl].unsqueeze(2).broadcast_to((P, Gt, 8, 4))
                # e1 and t2 share one tile so their pair-min runs as ONE op
                ET = xe.tile([P, Gt, 16, 4], F16, tag="ET")
                nc.vector.tensor_sub(ET[:, :, 8:16, :], a2bcT, agbT)
                l1d = xe.tile([P, Gt, 3], F16, tag="l1d")
                if KNOB.get("l1d_gp", True):
                    gp_tt(l1d, s["pst"], gta[:, sl, 3:6], ALU.subtract)
                else:
                    nc.vector.tensor_sub(l1d, s["pst"], gta[:, sl, 3:6])
                s["eeb"], s["ET"], s["l1d"] = eeb, ET, l1d

            def S4(t):
                s = st[t]
                r0, Gt = TILES[t]
                eeb, ET = s["eeb"], s["ET"]
                nc.vector.tensor_tensor(
                    ET[:, :, 0:8, :], eeb[:, :, :, 0:4], eeb[:, :, :, 4:8],
                    op=ALU.min,
                )
                ett = scr.tile([P, Gt, 16, 2], F16, tag="ett")
                nc.vector.tensor_tensor(
                    ett, ET[:, :, :, 0:2], ET[:, :, :, 2:4], op=ALU.min
                )
                St = scr.tile([P, Gt, 16], F16, tag="St")
                nc.vector.tensor_tensor(
                    St, ett[:, :, :, 0], ett[:, :, :, 1], op=ALU.min
                )
                junk16 = scr.tile([P, Gt, 16], F16, tag="junk16")
                nc.scalar.activation(
                    junk16, St,
                    ACTF.Identity, accum_out=accT[:, t, MINSUM : MINSUM + 1],
                )
                junk3 = scr.tile([P, Gt, 3], F16, tag="junk3")
                nc.scalar.activation(
                    junk3, s["l1d"], ACTF.Abs, accum_out=accT[:, t, L1 : L1 + 1]
                )
                s.clear()

            stages = [S0, S1, S2, S3, S4]
            nstage = len(stages)
            # phase-0 chunk 0 first (gates tiles 0..1); chunk 1 emitted two
            # steps in so its Pool work does not collide with tile-0's d-sum.
            # Within a step, emit S1 first: each stage's cross-engine
            # producers then finished a full step earlier, so no engine's
            # in-order queue blocks.
            # chunk schedule: (emit-step, which) pairs; chunk0 up front
            P0a(0)
            P0b(0)
            scheds = {
                "B": {1: [lambda: P0a(1), lambda: P0b(1)]},
                "C": {0: [lambda: P0a(1)], 1: [lambda: P0b(1)]},
                "D": {0: [lambda: P0a(1), lambda: P0b(1)]},
                "E": {2: [lambda: P0a(1), lambda: P0b(1)]},
                "3w": {1: [lambda: P0a(1)], 2: [lambda: P0b(1), lambda: P0a(2)],
                       3: [lambda: P0b(2)]},
                "3x": {0: [lambda: P0a(1)], 1: [lambda: P0b(1), lambda: P0a(2)],
                       2: [lambda: P0b(2)]},
            }
            sched = scheds[KNOB.get("sched", "B")]
            if True:
                for k in range(ntiles + nstage - 1):
                    for fn_ in sched.get(k, ()):
                        fn_()
                    sorder = (range(nstage - 1, -1, -1) if KNOB["order"] == "rev"
                              else range(nstage))
                    for sidx in sorder:
                        t = k - sidx
                        if 0 <= t < ntiles:
                            stages[sidx](t)

            nc.sync.dma_start(out=out, in_=accT.rearrange("p t x -> p (t x)"))

    nc.compile()
    return nc


_CACHE = {}


def _get_nc():
    if "nc" not in _CACHE:
        _CACHE["nc"] = build_nc()
    return _CACHE["nc"]


def combine_partials(outs):
    """outs: list of (P, ntiles*NSLOT) arrays -> (cd_sum, l1_sum) float64."""
    tot_min = 0.0
    tot_sqa = 0.0
    tot_sqb = 0.0
    tot_l1 = 0.0
    for o in outs:
        o = o.astype(np.float64).reshape(P, -1, NSLOT)
        tot_min += o[:, :, MINSUM].sum() + o[:, :, T2SUM].sum()
        tot_sqa += o[:, :, SQA].sum()
        tot_sqb += o[:, :, SQB].sum()
        tot_l1 += o[:, :, L1].sum()
    cd_sum = tot_min + 2.0 * tot_sqa + tot_sqb
    return cd_sum, tot_l1


def kernel(pred_pts, pred_size, gt_box):
    from concourse.bass_utils import run_bass_kernel_spmd

    pred_pts = np.asarray(pred_pts, dtype=np.float32)
    pred_size = np.asarray(pred_size, dtype=np.float32)
    gt_box = np.asarray(gt_box, dtype=np.float32)

    N = pred_pts.shape[0]
    assert N == N_TOTAL, f"expected {N_TOTAL} rows, got {N}"
    gt_flat = np.ascontiguousarray(gt_box.reshape(N, 12).astype(np.float16))
    pred = np.ascontiguousarray(pred_pts.reshape(N, 24).astype(np.float16))
    ps = np.ascontiguousarray(pred_size.astype(np.float16))

    in_maps = [
        {
            "gt": gt_flat[i * NC_N : (i + 1) * NC_N],
            "pred": pred[i * NC_N : (i + 1) * NC_N],
            "ps": ps[i * NC_N : (i + 1) * NC_N],
        }
        for i in range(N_CORES)
    ]
    res = run_bass_kernel_spmd(_get_nc(), in_maps, core_ids=list(range(N_CORES)))
    cd_sum, l1_sum = combine_partials([r["out"] for r in res.results])
    cd = cd_sum / (N * 8)
    l1 = l1_sum / (N * 3)
    return np.array([cd, l1], dtype=np.float32)
